# revision 4
# baseline (speedup 1.0000x reference)
"""MultiHeadAttention forward on 8 Trainium2 NeuronCores.

Sharding: core c in 0..7 handles batch b = c//4 and head-group hg = c%4
(4 of the 16 heads).  Each core computes its heads' attention and a
partial fc projection; the host sums the 4 partials per batch.

All matmuls run in float32r (TF32-like, ~2e-4 rounding, full PE rate at
N>=256).  The attention matrix is produced in BOTH orientations directly
on the PE (scores [q,k] for the softmax/attn output; scores^T [k,q] for
the attn@V contraction) which avoids transposing the 2048x2048 E matrix.
"""

import numpy as np

import concourse.bass as bass  # noqa: F401  (bass types referenced via bacc)
import concourse.mybir as mybir
from concourse import bacc
from concourse.tile import TileContext
from concourse.masks import make_identity
from concourse import bass_utils

F32 = mybir.dt.float32
F32R = mybir.dt.float32r
EXP = mybir.ActivationFunctionType.Exp

N_CORES = 8
B, S, DM = 2, 2048, 1024
N_HEAD, DK = 16, 64
HPC = 4                    # heads per core
HG_W = HPC * DK            # 256 weight columns per core
SCALE = 1.0 / np.sqrt(DK)  # folded into the exp activations

N_ST = S // 128            # 16 s(=k) tiles of 128
N_DMT = DM // 128          # 8 dm tiles
N_CHUNK = 4                # s-chunks of 512 for the projection stage
N_Q2 = 8                   # q-chunks of 256 for the attention stage


def build_bass(reps: int = 1):
    nc = bacc.Bacc("TRN2", target_bir_lowering=False, debug=False,
                   num_devices=N_CORES)
    q_d = nc.dram_tensor("qb", [S, DM], F32, kind="ExternalInput").ap()
    k_d = nc.dram_tensor("kb", [S, DM], F32, kind="ExternalInput").ap()
    v_d = nc.dram_tensor("vb", [S, DM], F32, kind="ExternalInput").ap()
    wq_d = nc.dram_tensor("wq", [DM, HG_W], F32, kind="ExternalInput").ap()
    wk_d = nc.dram_tensor("wk", [DM, HG_W], F32, kind="ExternalInput").ap()
    wv_d = nc.dram_tensor("wv", [DM, HG_W], F32, kind="ExternalInput").ap()
    wf_d = nc.dram_tensor("wfc", [HG_W, DM], F32, kind="ExternalInput").ap()
    attn_d = nc.dram_tensor("attn_o", [HPC, S, S], F32, kind="ExternalOutput").ap()
    out_d = nc.dram_tensor("out_o", [S, DM], F32, kind="ExternalOutput").ap()
    attn_f = attn_d.rearrange("h q k -> (h q) k")

    with TileContext(nc) as tc:
        with tc.tile_pool(name="const", bufs=1) as cp:
            ident_f = cp.tile([128, 128], F32, tag="ident_f")
            make_identity(nc, ident_f[:])
            ident_r = cp.tile([128, 128], F32R, tag="ident_r")
            nc.vector.tensor_copy(ident_r[:], ident_f[:])

            for _rep in range(reps):
                _body(nc, tc, q_d, k_d, v_d, wq_d, wk_d, wv_d, wf_d,
                      attn_f, out_d, ident_f, ident_r)
    nc.finalize()
    return nc


def _body(nc, tc, q_d, k_d, v_d, wq_d, wk_d, wv_d, wf_d, attn_f, out_d,
          ident_f, ident_r):
    with tc.tile_pool(name="persist", bufs=1) as pp:
        qhT = pp.tile([128, 2 * S], F32R, tag="qhT")   # [d'pair, pair*S + s]
        khT = pp.tile([128, 2 * S], F32R, tag="khT")
        vh = pp.tile([128, N_ST * HG_W], F32R, tag="vh")  # [k_in_tile, kt*256 + h*64 + d']
        wfc = pp.tile([128, 2 * DM], F32R, tag="wfc")  # [c_in_tile, ct*DM + dm]
        nc.gpsimd.dma_start(
            wfc[:].rearrange("p (t m) -> p t m", m=DM),
            wf_d.rearrange("(t p) m -> p t m", p=128))

        # ---- Stage TP: transpose q/k/v and project to qhT/khT/vh ----
        with (
            tc.tile_pool(name="wpool", bufs=1) as wp,
            tc.tile_pool(name="tp", bufs=1) as tp,
            tc.tile_pool(name="xload", bufs=2) as xl,
            tc.tile_pool(name="ps_tr", bufs=2, space="PSUM") as pstr,
            tc.tile_pool(name="ps_pr", bufs=2, space="PSUM") as pspr,
        ):
            w_sb = {}
            for name, wd in (("wq", wq_d), ("wk", wk_d), ("wv", wv_d)):
                w_sb[name] = wp.tile([128, N_DMT * HG_W], F32R, tag=name, name=f"w_{name}")
                nc.gpsimd.dma_start(
                    w_sb[name][:].rearrange("p (t m) -> p t m", m=HG_W),
                    wd.rearrange("(t p) m -> p t m", p=128))

            for chunk in range(N_CHUNK):
                xT = {}
                for name, xd in (("q", q_d), ("k", k_d), ("v", v_d)):
                    # load [512, 1024] as 4 partition-tiles, cast to f32r
                    xt = xl.tile([128, 4 * DM], F32R, tag="xload")
                    nc.gpsimd.dma_start(
                        xt[:].rearrange("p (st d) -> p st d", d=DM),
                        xd[chunk * 512:(chunk + 1) * 512, :]
                        .rearrange("(st p) d -> p st d", p=128))
                    # transpose into [dm, s] layout: col = dm*512 + st*128
                    xT[name] = tp.tile([128, N_DMT * 512], F32R, tag=f"{name}T", name=f"xT_{name}")
                    for st in range(4):
                        for dm in range(N_DMT):
                            pt = pstr.tile([128, 128], F32R, tag="pt")
                            nc.tensor.transpose(
                                pt[:], xt[:, st * DM + dm * 128:st * DM + (dm + 1) * 128],
                                ident_r[:])
                            nc.any.tensor_copy(
                                xT[name][:, dm * 512 + st * 128:dm * 512 + (st + 1) * 128],
                                pt[:])
                # qh^T / kh^T: out [d'pair 128, s 512]
                for wname, dstT, src in (("wq", qhT, "q"), ("wk", khT, "k")):
                    for pr in range(2):
                        pq = pspr.tile([128, 512], F32, tag="pproj")
                        for dm in range(N_DMT):
                            nc.tensor.matmul(
                                pq[:],
                                w_sb[wname][:, dm * HG_W + pr * 128:dm * HG_W + (pr + 1) * 128],
                                xT[src][:, dm * 512:(dm + 1) * 512],
                                start=(dm == 0), stop=(dm == N_DMT - 1))
                        nc.any.tensor_copy(
                            dstT[:, pr * S + chunk * 512:pr * S + (chunk + 1) * 512],
                            pq[:])
                # vh: out [s 128, d' 256] per s-subtile
                for st in range(4):
                    pv = pspr.tile([128, HG_W], F32, tag="pv")
                    for dm in range(N_DMT):
                        nc.tensor.matmul(
                            pv[:],
                            xT["v"][:, dm * 512 + st * 128:dm * 512 + (st + 1) * 128],
                            w_sb["wv"][:, dm * HG_W:(dm + 1) * HG_W],
                            start=(dm == 0), stop=(dm == N_DMT - 1))
                    kt = chunk * 4 + st
                    nc.any.tensor_copy(vh[:, kt * HG_W:(kt + 1) * HG_W], pv[:])

        # ---- Stage A: attention + fc ----
        with (
            tc.tile_pool(name="apool", bufs=2) as ap,
            tc.tile_pool(name="epool", bufs=2) as ep,
            tc.tile_pool(name="etpool", bufs=2) as etp,
            tc.tile_pool(name="attnp", bufs=2) as atp,
            tc.tile_pool(name="smallp", bufs=3) as sp,
            tc.tile_pool(name="ps_sc", bufs=2, space="PSUM") as pssc,
            tc.tile_pool(name="ps_st", bufs=2, space="PSUM") as psst,
            tc.tile_pool(name="ps_o", bufs=1, space="PSUM") as pso,
            tc.tile_pool(name="ps_fc", bufs=1, space="PSUM") as psfc,
        ):
            for q2 in range(N_Q2):
                ocatT = ap.tile([128, 2 * 256], F32R, tag="ocat")
                rcps = []
                for h in range(HPC):
                    pr, off = h // 2, (h % 2) * 64
                    base = pr * S
                    lq = qhT[off:off + 64, :]
                    lk = khT[off:off + 64, :]
                    sums2 = sp.tile([128, 2], F32, tag="sums")
                    E = []
                    # scores [q, k] + exp -> E, row sums
                    for qt in range(2):
                        qrow = q2 * 256 + qt * 128
                        Eq = ep.tile([128, S], F32R, tag="E")
                        parts = sp.tile([128, 2], F32, tag="parts")
                        for kc in range(2):
                            ps_s = pssc.tile([128, 1024], F32, tag="sc")
                            for nn in range(2):
                                kcol = kc * 1024 + nn * 512
                                nc.tensor.matmul(
                                    ps_s[:, nn * 512:(nn + 1) * 512],
                                    lq[:, base + qrow:base + qrow + 128],
                                    lk[:, base + kcol:base + kcol + 512],
                                    start=True, stop=True)
                            nc.scalar.activation(
                                Eq[:, kc * 1024:(kc + 1) * 1024], ps_s[:],
                                EXP, scale=float(SCALE),
                                accum_out=parts[:, kc:kc + 1])
                        nc.vector.tensor_add(
                            sums2[:, qt:qt + 1], parts[:, 0:1], parts[:, 1:2])
                        E.append(Eq)
                    # normalized attn out (fp32)
                    rcp2 = sp.tile([128, 2], F32, tag="rcp2")
                    nc.vector.reciprocal(rcp2[:], sums2[:])
                    for qt in range(2):
                        qrow = q2 * 256 + qt * 128
                        attn_t = atp.tile([128, S], F32, tag="attn")
                        nc.vector.tensor_scalar_mul(
                            attn_t[:], E[qt][:], rcp2[:, qt:qt + 1])
                        nc.sync.dma_start(
                            attn_f[h * S + qrow:h * S + qrow + 128, :], attn_t[:])
                    # reciprocal in free orientation for the o normalization
                    rbc = sp.tile([64, 256], F32, tag="rbc")
                    for qt in range(2):
                        ps_r = psst.tile([1, 128], F32, tag="st", name="ps_r")
                        nc.tensor.transpose(
                            ps_r[:], sums2[:, qt:qt + 1], ident_f[:])
                        rrow = sp.tile([1, 128], F32, tag="rrow", name="rrow")
                        nc.vector.reciprocal(rrow[:], ps_r[:])
                        nc.gpsimd.partition_broadcast(
                            rbc[:, qt * 128:(qt + 1) * 128], rrow[:])
                    # scores^T [k, q] + exp -> E^T
                    ET = etp.tile([128, N_ST * 256], F32R, tag="ET")
                    for kt in range(N_ST):
                        ps_t = psst.tile([128, 256], F32, tag="st")
                        nc.tensor.matmul(
                            ps_t[:],
                            lk[:, base + kt * 128:base + (kt + 1) * 128],
                            lq[:, base + q2 * 256:base + (q2 + 1) * 256],
                            start=True, stop=True)
                        nc.scalar.activation(
                            ET[:, kt * 256:(kt + 1) * 256], ps_t[:],
                            EXP, scale=float(SCALE))
                    # o^T = vh^T @ E^T  (accumulate over k tiles)
                    ps_oo = pso.tile([64, 256], F32, tag="o")
                    for kt in range(N_ST):
                        nc.tensor.matmul(
                            ps_oo[:],
                            vh[:, kt * HG_W + h * DK:kt * HG_W + (h + 1) * DK],
                            ET[:, kt * 256:(kt + 1) * 256],
                            start=(kt == 0), stop=(kt == N_ST - 1))
                    nc.vector.tensor_mul(
                        ocatT[off:off + 64, pr * 256:(pr + 1) * 256],
                        ps_oo[:], rbc[:])
                    rcps.append(rcp2)
                # fc for this q-chunk
                for st2 in range(2):
                    qrow = q2 * 256 + st2 * 128
                    ob = atp.tile([128, DM], F32, tag="ob")
                    for half in range(2):
                        ps_f = psfc.tile([128, 512], F32, tag="f")
                        for ct in range(2):
                            nc.tensor.matmul(
                                ps_f[:],
                                ocatT[:, ct * 256 + st2 * 128:ct * 256 + (st2 + 1) * 128],
                                wfc[:, ct * DM + half * 512:ct * DM + (half + 1) * 512],
                                start=(ct == 0), stop=(ct == 1))
                        nc.scalar.copy(ob[:, half * 512:(half + 1) * 512], ps_f[:])
                    nc.sync.dma_start(out_d[qrow:qrow + 128, :], ob[:])


_NC_CACHE = {}


def _get_nc(reps: int = 1):
    if reps not in _NC_CACHE:
        _NC_CACHE[reps] = build_bass(reps)
    return _NC_CACHE[reps]


def make_in_maps(q, k, v, w_qs, w_ks, w_vs, w_fc):
    in_maps = []
    for c in range(N_CORES):
        b, hg = divmod(c, 4)
        cs = slice(hg * HG_W, (hg + 1) * HG_W)
        in_maps.append({
            "qb": np.ascontiguousarray(q[b]),
            "kb": np.ascontiguousarray(k[b]),
            "vb": np.ascontiguousarray(v[b]),
            "wq": np.ascontiguousarray(w_qs[:, cs]),
            "wk": np.ascontiguousarray(w_ks[:, cs]),
            "wv": np.ascontiguousarray(w_vs[:, cs]),
            "wfc": np.ascontiguousarray(w_fc[cs, :]),
        })
    return in_maps


def assemble(results):
    attn = np.empty((B, N_HEAD, S, S), np.float32)
    out = np.zeros((B, S, DM), np.float32)
    for c in range(N_CORES):
        b, hg = divmod(c, 4)
        attn[b, hg * HPC:(hg + 1) * HPC] = results[c]["attn_o"]
        out[b] += results[c]["out_o"]
    return out, attn


def kernel(q, k, v, w_qs, w_ks, w_vs, w_fc):
    q = np.asarray(q, np.float32)
    k = np.asarray(k, np.float32)
    v = np.asarray(v, np.float32)
    nc = _get_nc(1)
    in_maps = make_in_maps(q, k, v,
                           np.asarray(w_qs, np.float32),
                           np.asarray(w_ks, np.float32),
                           np.asarray(w_vs, np.float32),
                           np.asarray(w_fc, np.float32))
    res = bass_utils.run_bass_kernel_spmd(nc, in_maps,
                                          core_ids=list(range(N_CORES)))
    return assemble(res.results)


# revision 18
# speedup vs baseline: 2.4999x; 2.4999x over previous
"""MultiHeadAttention forward on 8 Trainium2 NeuronCores.

Sharding: core c in 0..7 handles batch b = c//4 and head-group hg = c%4
(4 of the 16 heads).  Each core computes its heads' attention and a
partial fc projection; the host sums the 4 partials per batch.

All matmuls run in float32r (TF32-like, ~2e-4 rounding, full PE rate at
N>=256).  The attention matrix is produced in BOTH orientations directly
on the PE (scores [q,k] for the softmax/attn output; scores^T [k,q] for
the attn@V contraction) which avoids transposing the 2048x2048 E matrix.
"""

import numpy as np

import concourse.bass as bass  # noqa: F401  (bass types referenced via bacc)
import concourse.mybir as mybir
from concourse import bacc
from concourse.tile import TileContext
from concourse.masks import make_identity
from concourse import bass_utils

F32 = mybir.dt.float32
F32R = mybir.dt.float32r
EXP = mybir.ActivationFunctionType.Exp

N_CORES = 8
B, S, DM = 2, 2048, 1024
N_HEAD, DK = 16, 64
HPC = 4                    # heads per core
HG_W = HPC * DK            # 256 weight columns per core
SCALE = 1.0 / np.sqrt(DK)  # folded into the exp activations

import os
BIG_ACT = os.environ.get("KV_BIG_ACT", "1") == "1"
PE_BCAST = os.environ.get("KV_PE_BCAST", "1") == "1"
STAGE = os.environ.get("KV_STAGE", "full")  # full | tp | noscT | noscA
SC2 = os.environ.get("KV_SC2", "0") == "1"    # split exp-A into 2x[128,1024], sc bufs=2
O_COPY = os.environ.get("KV_O_COPY", "0") == "1"  # copy attnV psum out before normalize-mul
BUFS3 = os.environ.get("KV_BUFS3", "0") == "1"  # deeper SBUF pools
PS3 = os.environ.get("KV_PS3", "1") == "1"      # psst bufs=3, fc shares st tag
LAZY_NORM = os.environ.get("KV_LAZY_NORM", "0") == "1"  # raw o copy; one mul per q2
SYNC_LOADS = os.environ.get("KV_SYNC_LOADS", "0") == "1"
NO_ONORM = os.environ.get("KV_NO_ONORM", "0") == "1"

N_ST = S // 128            # 16 s(=k) tiles of 128
N_DMT = DM // 128          # 8 dm tiles
N_CHUNK = 4                # s-chunks of 512 for the projection stage
N_Q2 = 8                   # q-chunks of 256 for the attention stage


def build_bass(reps: int = 1, timing: bool = False, const_inputs=None, loop_n: int = 1):
    nc = bacc.Bacc("TRN2", target_bir_lowering=False, debug=False,
                   num_devices=N_CORES)
    if const_inputs is not None:
        def mkin(name, shape):
            return nc.inline_tensor(
                np.ascontiguousarray(const_inputs[name], np.float32),
                name=name).ap()
    else:
        def mkin(name, shape):
            return nc.dram_tensor(name, shape, F32, kind="ExternalInput").ap()
    q_d = mkin("qb", [S, DM])
    k_d = mkin("kb", [S, DM])
    v_d = mkin("vb", [S, DM])
    wq_d = mkin("wq", [DM, HG_W])
    wk_d = mkin("wk", [DM, HG_W])
    wv_d = mkin("wv", [DM, HG_W])
    wf_d = mkin("wfc", [HG_W, DM])
    okind = "Internal" if timing else "ExternalOutput"
    attn_d = nc.dram_tensor("attn_o", [HPC, S, S], F32, kind=okind).ap()
    out_d = nc.dram_tensor("out_o", [S, DM], F32, kind=okind).ap()
    attn_f = attn_d.rearrange("h q k -> (h q) k")
    sink_d = None
    if timing:
        sink_d = nc.dram_tensor("sink_o", [1, 4], F32, kind="ExternalOutput").ap()

    with TileContext(nc) as tc:
        with tc.tile_pool(name="const", bufs=1) as cp:
            ident_f = cp.tile([128, 128], F32, tag="ident_f")
            make_identity(nc, ident_f[:])
            ident_r = cp.tile([128, 128], F32R, tag="ident_r")
            nc.vector.tensor_copy(ident_r[:], ident_f[:])
            ones_f = cp.tile([1, 64], F32, tag="ones_f")
            nc.gpsimd.memset(ones_f[:], 1.0)
            ones_r = cp.tile([1, 64], F32R, tag="ones_r")
            nc.vector.tensor_copy(ones_r[:], ones_f[:])

            if loop_n > 1:
                with tc.For_i(0, loop_n, 1):
                    _body(nc, tc, q_d, k_d, v_d, wq_d, wk_d, wv_d, wf_d,
                          attn_f, out_d, ident_f, ident_r, ones_f)
            else:
                for _rep in range(reps):
                    _body(nc, tc, q_d, k_d, v_d, wq_d, wk_d, wv_d, wf_d,
                          attn_f, out_d, ident_f, ident_r, ones_f)
            if timing:
                with tc.tile_pool(name="sinkp", bufs=1) as skp:
                    sink_t = skp.tile([1, 4], F32, tag="sink")
                    nc.gpsimd.memset(sink_t[:], 1.0)
                    nc.sync.dma_start(sink_d[:], sink_t[:])
    nc.finalize()
    return nc


def _body(nc, tc, q_d, k_d, v_d, wq_d, wk_d, wv_d, wf_d, attn_f, out_d,
          ident_f, ident_r, ones_f):
    with tc.tile_pool(name="persist", bufs=1) as pp:
        qhT = pp.tile([128, 2 * S], F32R, tag="qhT")   # [d'pair, pair*S + s]
        khT = pp.tile([128, 2 * S], F32R, tag="khT")
        vh = pp.tile([128, N_ST * HG_W], F32R, tag="vh")  # [k_in_tile, kt*256 + h*64 + d']
        wfc = pp.tile([128, 2 * DM], F32R, tag="wfc")  # [c_in_tile, ct*DM + dm]
        nc.gpsimd.dma_start(
            wfc[:].rearrange("p (t m) -> p t m", m=DM),
            wf_d.rearrange("(t p) m -> p t m", p=128))

        # ---- Stage TP: transpose q/k/v and project to qhT/khT/vh ----
        with (
            tc.tile_pool(name="wpool", bufs=1) as wp,
            tc.tile_pool(name="tp", bufs=1) as tp,
            tc.tile_pool(name="xload", bufs=2) as xl,
            tc.tile_pool(name="ps_tr", bufs=2, space="PSUM") as pstr,
            tc.tile_pool(name="ps_pr", bufs=2, space="PSUM") as pspr,
        ):
            w_sb = {}
            for name, wd in (("wq", wq_d), ("wk", wk_d), ("wv", wv_d)):
                w_sb[name] = wp.tile([128, N_DMT * HG_W], F32R, tag=name, name=f"w_{name}")
                if SYNC_LOADS:
                    w_f = wp.tile([128, N_DMT * HG_W], F32, tag=name + "f", name=f"wf_{name}")
                    nc.sync.dma_start(
                        w_f[:].rearrange("p (t m) -> p t m", m=HG_W),
                        wd.rearrange("(t p) m -> p t m", p=128))
                    nc.any.tensor_copy(w_sb[name][:], w_f[:])
                else:
                    nc.gpsimd.dma_start(
                        w_sb[name][:].rearrange("p (t m) -> p t m", m=HG_W),
                        wd.rearrange("(t p) m -> p t m", p=128))

            for chunk in range(N_CHUNK):
                xT = {}
                for name, xd in (("q", q_d), ("k", k_d), ("v", v_d)):
                    # load [512, 1024] as 4 partition-tiles, cast to f32r
                    xdt = F32 if SYNC_LOADS else F32R
                    xt = xl.tile([128, 4 * DM], xdt, tag="xload")
                    dma_eng = nc.sync if SYNC_LOADS else nc.gpsimd
                    dma_eng.dma_start(
                        xt[:].rearrange("p (st d) -> p st d", d=DM),
                        xd[chunk * 512:(chunk + 1) * 512, :]
                        .rearrange("(st p) d -> p st d", p=128))
                    # transpose into [dm, s] layout: col = dm*512 + st*128
                    xT[name] = tp.tile([128, N_DMT * 512], F32R, tag=f"{name}T", name=f"xT_{name}")
                    for st in range(4):
                        for dm in range(N_DMT):
                            pt = pstr.tile([128, 128], F32 if SYNC_LOADS else F32R, tag="pt")
                            nc.tensor.transpose(
                                pt[:], xt[:, st * DM + dm * 128:st * DM + (dm + 1) * 128],
                                ident_f[:] if SYNC_LOADS else ident_r[:])
                            nc.vector.tensor_copy(
                                xT[name][:, dm * 512 + st * 128:dm * 512 + (st + 1) * 128],
                                pt[:])
                # qh^T / kh^T: out [d'pair 128, s 512]
                for wname, dstT, src in (("wq", qhT, "q"), ("wk", khT, "k")):
                    for pr in range(2):
                        pq = pspr.tile([128, 512], F32, tag="pproj")
                        for dm in range(N_DMT):
                            nc.tensor.matmul(
                                pq[:],
                                w_sb[wname][:, dm * HG_W + pr * 128:dm * HG_W + (pr + 1) * 128],
                                xT[src][:, dm * 512:(dm + 1) * 512],
                                start=(dm == 0), stop=(dm == N_DMT - 1))
                        nc.vector.tensor_copy(
                            dstT[:, pr * S + chunk * 512:pr * S + (chunk + 1) * 512],
                            pq[:])
                # vh: out [s 128, d' 256] per s-subtile
                for st in range(4):
                    pv = pspr.tile([128, HG_W], F32, tag="pv")
                    for dm in range(N_DMT):
                        nc.tensor.matmul(
                            pv[:],
                            xT["v"][:, dm * 512 + st * 128:dm * 512 + (st + 1) * 128],
                            w_sb["wv"][:, dm * HG_W:(dm + 1) * HG_W],
                            start=(dm == 0), stop=(dm == N_DMT - 1))
                    kt = chunk * 4 + st
                    nc.vector.tensor_copy(vh[:, kt * HG_W:(kt + 1) * HG_W], pv[:])

        if STAGE == "tp":
            return
        # ---- Stage A: attention + fc ----
        with (
            tc.tile_pool(name="apool", bufs=2) as ap,
            tc.tile_pool(name="epool", bufs=(3 if BUFS3 else 2)) as ep,
            tc.tile_pool(name="etpool", bufs=(3 if BUFS3 else 2)) as etp,
            tc.tile_pool(name="attnp", bufs=(3 if BUFS3 else 2)) as atp,
            tc.tile_pool(name="smallp", bufs=(6 if BUFS3 else 3)) as sp,
            tc.tile_pool(name="ps_sc", bufs=(2 if (SC2 or not BIG_ACT) else 1), space="PSUM") as pssc,
            tc.tile_pool(name="ps_st", bufs=(3 if PS3 else 2), space="PSUM") as psst,
            tc.tile_pool(name="ps_o", bufs=1, space="PSUM") as pso,
        ):
          from contextlib import ExitStack as _ES
          with _ES() as _es:
            psfc = None if PS3 else _es.enter_context(
                tc.tile_pool(name="ps_fc", bufs=1, space="PSUM"))
            for q2 in range(N_Q2):
                ocatT = ap.tile([128, 2 * 256], F32R, tag="ocat")
                ps_rbc = psst.tile([128, 512], F32, tag="st", name="ps_rbc") \
                    if LAZY_NORM else None
                rcps = []
                for h in range(HPC):
                    pr, off = h // 2, (h % 2) * 64
                    base = pr * S
                    lq = qhT[off:off + 64, :]
                    lk = khT[off:off + 64, :]
                    sums2 = sp.tile([128, 2], F32, tag="sums")
                    if STAGE == "noscA":
                        nc.gpsimd.memset(sums2[:], 2.0)
                    E = []
                    # scores [q, k] + exp -> E, row sums
                    for qt in range(2) if STAGE != "noscA" else []:
                        qrow = q2 * 256 + qt * 128
                        Eq = ep.tile([128, S], F32R, tag="E")
                        if BIG_ACT and not SC2:
                            ps_s = pssc.tile([128, 2048], F32, tag="sc")
                            for nn in range(4):
                                kcol = nn * 512
                                nc.tensor.matmul(
                                    ps_s[:, kcol:kcol + 512],
                                    lq[:, base + qrow:base + qrow + 128],
                                    lk[:, base + kcol:base + kcol + 512],
                                    start=True, stop=True)
                            nc.scalar.activation(
                                Eq[:], ps_s[:], EXP, scale=float(SCALE),
                                accum_out=sums2[:, qt:qt + 1])
                        else:
                            parts = sp.tile([128, 2], F32, tag="parts")
                            for kc in range(2):
                                ps_s = pssc.tile([128, 1024], F32, tag="sc")
                                for nn in range(2):
                                    kcol = kc * 1024 + nn * 512
                                    nc.tensor.matmul(
                                        ps_s[:, nn * 512:(nn + 1) * 512],
                                        lq[:, base + qrow:base + qrow + 128],
                                        lk[:, base + kcol:base + kcol + 512],
                                        start=True, stop=True)
                                nc.scalar.activation(
                                    Eq[:, kc * 1024:(kc + 1) * 1024], ps_s[:],
                                    EXP, scale=float(SCALE),
                                    accum_out=parts[:, kc:kc + 1])
                            nc.vector.tensor_add(
                                sums2[:, qt:qt + 1], parts[:, 0:1], parts[:, 1:2])
                        E.append(Eq)
                    # normalized attn out (fp32)
                    rcp2 = sp.tile([128, 2], F32, tag="rcp2")
                    nc.vector.reciprocal(rcp2[:], sums2[:])
                    for qt in range(2) if STAGE != "noscA" else []:
                        qrow = q2 * 256 + qt * 128
                        attn_t = atp.tile([128, S], F32, tag="attn")
                        nc.vector.tensor_scalar_mul(
                            attn_t[:], E[qt][:], rcp2[:, qt:qt + 1])
                        nc.sync.dma_start(
                            attn_f[h * S + qrow:h * S + qrow + 128, :], attn_t[:])
                    # reciprocal in free orientation for the o normalization
                    if LAZY_NORM and not NO_ONORM:
                        for qt in range(2):
                            ps_r = psst.tile([1, 128], F32, tag="st", name="ps_r")
                            nc.tensor.transpose(
                                ps_r[:], sums2[:, qt:qt + 1], ident_f[:])
                            rrow = sp.tile([1, 128], F32, tag="rrow", name="rrow")
                            nc.vector.reciprocal(rrow[:], ps_r[:])
                            nc.tensor.matmul(
                                ps_rbc[off:off + 64,
                                       pr * 256 + qt * 128:pr * 256 + (qt + 1) * 128],
                                ones_f[:], rrow[:],
                                start=True, stop=True,
                                skip_group_check=True)
                        rbc = None
                    elif PE_BCAST and not NO_ONORM:
                        ps_b = psst.tile([64, 256], F32, tag="st", name="ps_b")
                        for qt in range(2):
                            ps_r = psst.tile([1, 128], F32, tag="st", name="ps_r")
                            nc.tensor.transpose(
                                ps_r[:], sums2[:, qt:qt + 1], ident_f[:])
                            rrow = sp.tile([1, 128], F32, tag="rrow", name="rrow")
                            nc.vector.reciprocal(rrow[:], ps_r[:])
                            nc.tensor.matmul(
                                ps_b[:, qt * 128:(qt + 1) * 128],
                                ones_f[:], rrow[:],
                                start=True, stop=True,
                                skip_group_check=True)
                        rbc = sp.tile([64, 256], F32, tag="rbc")
                        nc.vector.tensor_copy(rbc[:], ps_b[:])
                    else:
                        rbc = sp.tile([64, 256], F32, tag="rbc")
                        if not NO_ONORM:
                            for qt in range(2):
                                ps_r = psst.tile([1, 128], F32, tag="st", name="ps_r")
                                nc.tensor.transpose(
                                    ps_r[:], sums2[:, qt:qt + 1], ident_f[:])
                                rrow = sp.tile([1, 128], F32, tag="rrow", name="rrow")
                                nc.vector.reciprocal(rrow[:], ps_r[:])
                                nc.gpsimd.partition_broadcast(
                                    rbc[:, qt * 128:(qt + 1) * 128], rrow[:])
                    # scores^T [k, q] + exp -> E^T
                    if STAGE == "noscT":
                        continue
                    ET = etp.tile([128, N_ST * 256], F32R, tag="ET")
                    if BIG_ACT:
                        for kp in range(N_ST // 2):
                            ps_t = psst.tile([128, 512], F32, tag="st")
                            for kk in range(2):
                                kt = kp * 2 + kk
                                nc.tensor.matmul(
                                    ps_t[:, kk * 256:(kk + 1) * 256],
                                    lk[:, base + kt * 128:base + (kt + 1) * 128],
                                    lq[:, base + q2 * 256:base + (q2 + 1) * 256],
                                    start=(kk == 0), stop=(kk == 1),
                                    skip_group_check=True)
                            nc.scalar.activation(
                                ET[:, kp * 512:(kp + 1) * 512], ps_t[:],
                                EXP, scale=float(SCALE))
                    else:
                        for kt in range(N_ST):
                            ps_t = psst.tile([128, 256], F32, tag="st")
                            nc.tensor.matmul(
                                ps_t[:],
                                lk[:, base + kt * 128:base + (kt + 1) * 128],
                                lq[:, base + q2 * 256:base + (q2 + 1) * 256],
                                start=True, stop=True)
                            nc.scalar.activation(
                                ET[:, kt * 256:(kt + 1) * 256], ps_t[:],
                                EXP, scale=float(SCALE))
                    # o^T = vh^T @ E^T  (accumulate over k tiles)
                    ps_oo = pso.tile([64, 256], F32, tag="o")
                    for kt in range(N_ST):
                        nc.tensor.matmul(
                            ps_oo[:],
                            vh[:, kt * HG_W + h * DK:kt * HG_W + (h + 1) * DK],
                            ET[:, kt * 256:(kt + 1) * 256],
                            start=(kt == 0), stop=(kt == N_ST - 1))
                    if LAZY_NORM and not NO_ONORM:
                        nc.vector.tensor_copy(
                            ocatT[off:off + 64, pr * 256:(pr + 1) * 256], ps_oo[:])
                    elif NO_ONORM:
                        nc.vector.tensor_copy(
                            ocatT[off:off + 64, pr * 256:(pr + 1) * 256], ps_oo[:])
                    elif O_COPY:
                        o_sb = sp.tile([64, 256], F32, tag="osb")
                        nc.vector.tensor_copy(o_sb[:], ps_oo[:])
                        nc.vector.tensor_mul(
                            ocatT[off:off + 64, pr * 256:(pr + 1) * 256],
                            o_sb[:], rbc[:])
                    else:
                        nc.vector.tensor_mul(
                            ocatT[off:off + 64, pr * 256:(pr + 1) * 256],
                            ps_oo[:], rbc[:])
                    rcps.append(rcp2)
                if LAZY_NORM and not NO_ONORM and STAGE != "noscT":
                    nc.vector.tensor_mul(ocatT[:], ocatT[:], ps_rbc[:])
                # fc for this q-chunk
                for st2 in range(2) if STAGE != "noscT" else []:
                    qrow = q2 * 256 + st2 * 128
                    ob = atp.tile([128, DM], F32, tag="ob")
                    for half in range(2):
                        ps_f = psst.tile([128, 512], F32, tag="st", name="ps_f") if PS3 \
                            else psfc.tile([128, 512], F32, tag="f")
                        for ct in range(2):
                            nc.tensor.matmul(
                                ps_f[:],
                                ocatT[:, ct * 256 + st2 * 128:ct * 256 + (st2 + 1) * 128],
                                wfc[:, ct * DM + half * 512:ct * DM + (half + 1) * 512],
                                start=(ct == 0), stop=(ct == 1))
                        nc.vector.tensor_copy(ob[:, half * 512:(half + 1) * 512], ps_f[:])
                    nc.sync.dma_start(out_d[qrow:qrow + 128, :], ob[:])


_NC_CACHE = {}


def _get_nc(reps: int = 1):
    if reps not in _NC_CACHE:
        _NC_CACHE[reps] = build_bass(reps)
    return _NC_CACHE[reps]


def make_in_maps(q, k, v, w_qs, w_ks, w_vs, w_fc):
    in_maps = []
    for c in range(N_CORES):
        b, hg = divmod(c, 4)
        cs = slice(hg * HG_W, (hg + 1) * HG_W)
        in_maps.append({
            "qb": np.ascontiguousarray(q[b]),
            "kb": np.ascontiguousarray(k[b]),
            "vb": np.ascontiguousarray(v[b]),
            "wq": np.ascontiguousarray(w_qs[:, cs]),
            "wk": np.ascontiguousarray(w_ks[:, cs]),
            "wv": np.ascontiguousarray(w_vs[:, cs]),
            "wfc": np.ascontiguousarray(w_fc[cs, :]),
        })
    return in_maps


def assemble(results):
    attn = np.empty((B, N_HEAD, S, S), np.float32)
    out = np.zeros((B, S, DM), np.float32)
    for c in range(N_CORES):
        b, hg = divmod(c, 4)
        attn[b, hg * HPC:(hg + 1) * HPC] = results[c]["attn_o"]
        out[b] += results[c]["out_o"]
    return out, attn


def kernel(q, k, v, w_qs, w_ks, w_vs, w_fc):
    q = np.asarray(q, np.float32)
    k = np.asarray(k, np.float32)
    v = np.asarray(v, np.float32)
    nc = _get_nc(1)
    in_maps = make_in_maps(q, k, v,
                           np.asarray(w_qs, np.float32),
                           np.asarray(w_ks, np.float32),
                           np.asarray(w_vs, np.float32),
                           np.asarray(w_fc, np.float32))
    res = bass_utils.run_bass_kernel_spmd(nc, in_maps,
                                          core_ids=list(range(N_CORES)))
    return assemble(res.results)
